# revision 1
# baseline (speedup 1.0000x reference)
"""Trainium2 Bass kernel for nn_CustomTransformer_60619168416497.

kernel(**inputs) takes the FULL unsharded inputs (as produced by
setup_inputs()) and returns the FULL output (scalar f32 loss), running the
heavy X-dependent work on 8 NeuronCores (data parallel over the batch).

-- Algebraic reduction -------------------------------------------------------
Only h_2[:, -1] (the cls row) reaches the output head, so the attention never
needs materializing. Folding the tiny weights on the host:
    w     = W1 @ W_k @ (cls@W_q) / sqrt(32)     [8]
    N     = W1 @ W_v @ W2                       [8,2]
    a_cls = cls . (W_k @ (cls@W_q))/sqrt(32)    scalar
Per batch b (normalized x = (X - mu)/sigma'), the 257-way softmax needs only
    M    = max_j alpha*t_j          (t_j = X[b,j,:] @ w)
    den  = sum_j exp(alpha*t_j - M)
    G2_c = sum_j exp(alpha*t_j - M) * (X[b,j,:] @ N[:,c])
from which the host recovers z[b] and the NLL in closed form (f64).  mu and
sigma are global scalars over all of X; the host computes them exactly in f64
during input prep (prep already touches every element for the transpose/cast),
so a single device launch suffices.

-- Device work (per core, 256 batches, ONE launch) ---------------------------
Packed layout: planes P[i*16+u, v*256+j] = bf16(alpha*w_i * X[b,j,i]) with
local batch b = u*16+v.  A single stationary [128,48] computes all three
per-token contractions in ONE PE pass over the 4096 columns (each X element
streams through the PE exactly once):
    psum[q*16+u, v*256+j],  q=0: alpha*t (coeff 1), q=1/2: r_c (coeff N_c/aw)
A PSUM->SBUF copy then an SBUF->SBUF "bridge" DMA regroups partitions
(u, col-chunk g) -> partition u*8+g so softmax post-ops (max / Exp+accum /
mul+reduce) run at full 128-lane occupancy on [128, 512] tiles.  Out: per
batch M, den, G2 -> [128, 8] f32; host finishes the loss in f64.
The NEFF is input-independent, so compilation caches across calls.
"""
import numpy as np
import ml_dtypes

import concourse.tile as tile
import concourse.mybir as mybir
from concourse import bacc
from concourse.bass_utils import run_bass_kernel_spmd

F32 = mybir.dt.float32
BF16 = mybir.dt.bfloat16
NCORES = 8
BPC = 256          # batches per core
L = 256            # tokens
I = 8              # features
H = 32
EPS = 1e-7
PCOLS = 4096       # v*256 + j

bf16 = ml_dtypes.bfloat16


# ---------------------------------------------------------------- host math
def _fold_weights(W1, cls_tok, W_q, W_k, W_v, W_t, W2):
    f8 = np.float64
    W1, cls_tok, W_q, W_k, W_v, W_t, W2 = [np.asarray(a, f8) for a in
                                           (W1, cls_tok, W_q, W_k, W_v, W_t, W2)]
    u = (W_k @ (cls_tok @ W_q)) / np.sqrt(f8(H))
    w = W1 @ u
    N = (W1 @ W_v) @ W2
    return dict(
        w=w, N=N,
        a_cls=float(cls_tok @ u),
        sumw=float(w.sum()),
        n1=N.sum(axis=0),
        v2=(cls_tok @ W_v) @ W2,
        t2=(cls_tok @ W_t) @ W2,
    )


def _host_stats(X):
    Xd = np.asarray(X, np.float64)
    mu = Xd.mean()
    sigma = Xd.std(ddof=1) + EPS
    return float(mu), float(sigma), float(1.0 / sigma)


def _prep_inputs(X, coef, Wst):
    """Per-core packed input [128, 48 + 4096]: stationary cols then planes
    (i,u) x (v,j), planes scaled by coef[i]."""
    X = np.asarray(X, np.float32)
    per_core = []
    for c in range(NCORES):
        xc = X[c * BPC:(c + 1) * BPC].reshape(16, 16, L, I)     # [u, v, j, i]
        a = (xc * coef[None, None, None, :]).astype(bf16)
        planes = np.ascontiguousarray(a.transpose(3, 0, 1, 2)).reshape(128, PCOLS)
        per_core.append(np.concatenate([Wst, planes], axis=1))
    return per_core


def _build_stationary(c0, c1, c2):
    """Wst[i*16+u, q*16+u] = cq[i]  (q=0: t, q=1: r0, q=2: r1)."""
    Wst = np.zeros((128, 48), np.float32)
    iu = np.arange(128)
    i_idx, u_idx = iu // 16, iu % 16
    Wst[iu, u_idx] = c0[i_idx]
    Wst[iu, 16 + u_idx] = c1[i_idx]
    Wst[iu, 32 + u_idx] = c2[i_idx]
    return Wst.astype(bf16)


# ---------------------------------------------------------------- device body
def _main_body(nc):
    xp = nc.dram_tensor("xp", [128, 48 + PCOLS], BF16, kind="ExternalInput")
    outd = nc.dram_tensor("out", [128, 8], F32, kind="ExternalOutput")

    with tile.TileContext(nc) as tc:
        with (
            tc.tile_pool(name="xpool", bufs=1) as xpool,
            tc.tile_pool(name="ps", bufs=1, space="PSUM") as ps,
            tc.tile_pool(name="work", bufs=1) as work,
            tc.tile_pool(name="outp", bufs=1) as outp,
        ):
            # chunk0 carries the stationary (cols 0:48) + first 2 matmul blocks
            xt = [xpool.tile([128, 1072 if k == 0 else 1024], BF16,
                             name=f"x{k}", tag=f"x{k}") for k in range(4)]
            for k in range(4):
                eng = nc.sync if k % 2 == 0 else nc.scalar
                lo = 0 if k == 0 else 48 + k * 1024
                eng.dma_start(xt[k][:], xp[:, lo:48 + (k + 1) * 1024])
            wt = xt[0][:, 0:48]

            # one PSUM tile spanning all 8 banks; 8 single-pass matmuls
            pt = ps.tile([48, PCOLS], F32, name="pt", tag="pt")
            for k in range(8):
                off = 48 if k // 2 == 0 else 0
                nc.tensor.matmul(pt[:, k * 512:(k + 1) * 512], wt,
                                 xt[k // 2][:, off + (k % 2) * 512:
                                            off + (k % 2) * 512 + 512],
                                 start=True, stop=True, skip_group_check=True)

            # PSUM -> SBUF staging (bf16), chunk-wise on ACT/DVE/Pool
            st = work.tile([48, PCOLS], BF16, name="st", tag="st")
            for k in range(8):
                sl = slice(k * 512, (k + 1) * 512)
                if k % 2 == 0:
                    nc.scalar.copy(st[:, sl], pt[:, sl])
                else:
                    nc.vector.tensor_copy(st[:, sl], pt[:, sl])

            # bridge: [48, 4096] -> t3 [128, 1536]  (t | r0 | r1 slots)
            # dst partition u*8+g <- src (row q*16+u, col-chunk g)
            t3 = work.tile([128, 1536], BF16, name="t3", tag="t3")
            engs = (nc.sync, nc.scalar, nc.sync)
            for q in range(3):
                engs[q].dma_start(
                    t3[:, q * 512:(q + 1) * 512],
                    st[q * 16:(q + 1) * 16, :].rearrange("u (g c) -> u g c", g=8))

            out = outp.tile([128, 8], F32, name="out", tag="out")
            negaM = work.tile([128, 2], F32, name="negaM", tag="negaM")
            e = work.tile([128, 512], BF16, name="e", tag="e")

            # negaM = -max_j t  (fused negate); host recovers M = -out[:,0:2]
            nc.vector.tensor_reduce(
                negaM[:], t3[:, 0:512].rearrange("p (b j) -> p b j", b=2),
                axis=mybir.AxisListType.X, op=mybir.AluOpType.max, negate=True)
            nc.vector.tensor_copy(out[:, 0:2], negaM[:])
            for h in range(2):
                sl = slice(h * 256, (h + 1) * 256)
                nc.scalar.activation(e[:, sl], t3[:, sl],
                                     mybir.ActivationFunctionType.Exp,
                                     bias=negaM[:, h:h + 1],
                                     accum_out=out[:, 2 + h:3 + h])

            scr = work.tile([128, 1024], BF16, name="scr", tag="scr")
            nc.vector.tensor_mul(scr[:, 0:512], e[:], t3[:, 512:1024])
            nc.gpsimd.tensor_mul(scr[:, 512:1024], e[:], t3[:, 1024:1536])
            nc.vector.tensor_reduce(
                out[:, 4:8], scr[:].rearrange("p (s j) -> p s j", s=4),
                axis=mybir.AxisListType.X, op=mybir.AluOpType.add)
            nc.sync.dma_start(outd[:], out[:])
    return nc


# ---------------------------------------------------------------- host finish
def _host_finish(outs, fold, mu, alpha, y):
    O = np.stack([np.asarray(o, np.float64) for o in outs])   # [8, 128, 8]
    # batch order: (core, u, g, half) = core*256 + u*16 + 2g + half
    A = O.reshape(NCORES, 16, 8, 8)
    M = -A[..., 0:2].reshape(-1)          # device ships negaM
    den = A[..., 2:4].reshape(-1)
    G2 = np.stack([A[..., 4:6].reshape(-1), A[..., 6:8].reshape(-1)], axis=1)
    a_cls, sumw, n1, v2, t2 = (fold["a_cls"], fold["sumw"], fold["n1"],
                               fold["v2"], fold["t2"])
    l_shift = M - alpha * mu * sumw
    m_full = np.maximum(l_shift, a_cls)
    scale_tok = np.exp(l_shift - m_full)
    e_cls = np.exp(a_cls - m_full)
    denom = den * scale_tok + e_cls
    S_cls = e_cls / denom
    gN = G2 * scale_tok[:, None] / denom[:, None]
    z = (gN - (mu * (1.0 - S_cls))[:, None] * n1[None, :]) * alpha \
        + S_cls[:, None] * v2[None, :] + t2[None, :]
    zmax = z.max(axis=1)
    lse = zmax + np.log(np.exp(z[:, 0] - zmax) + np.exp(z[:, 1] - zmax))
    y = np.asarray(y).astype(np.int64).reshape(-1)
    zy = np.take_along_axis(z, y[:, None], axis=1)[:, 0]
    return (lse - zy).mean()


# ---------------------------------------------------------------- entry point
_NC_CACHE = {}


def _get_nc():
    if "main" not in _NC_CACHE:
        nc = bacc.Bacc("TRN2", target_bir_lowering=False, debug=False,
                       num_devices=NCORES)
        _main_body(nc)
        nc.compile()
        _NC_CACHE["main"] = nc
    return _NC_CACHE["main"]


def kernel(X, y, W1, cls_tok, W_q, W_k, W_v, W_t, W2):
    fold = _fold_weights(W1, cls_tok, W_q, W_k, W_v, W_t, W2)
    mu, sigma, alpha = _host_stats(X)
    w, N = fold["w"], fold["N"]
    aw = alpha * w
    if np.abs(w).min() >= 1e-3 * max(np.abs(w).max(), 1.0):
        # pre-scaled planes: ONE bf16 rounding on the exp-sensitive t path
        Wst = _build_stationary(np.ones(I, np.float64), N[:, 0] / aw, N[:, 1] / aw)
        per_core = _prep_inputs(X, aw.astype(np.float64), Wst)
    else:
        # near-zero w entry: raw planes, coefficients in the stationary
        Wst = _build_stationary(aw, N[:, 0], N[:, 1])
        per_core = _prep_inputs(X, np.ones(I, np.float64), Wst)

    nc = _get_nc()
    ins = [{"xp": p} for p in per_core]
    res = run_bass_kernel_spmd(nc, ins, core_ids=list(range(NCORES)))
    loss = _host_finish([r["out"] for r in res.results], fold, mu, alpha, y)
    return np.float32(loss)

